# revision 6
# baseline (speedup 1.0000x reference)
"""Trainium2 Bass kernel for DriverNet: 2-layer LSTM cell (single step, zero
initial state) + linear head over B=1M rows, data-parallel on 8 NeuronCores.

v2 redesign (vs the 68-74us baseline):
- x is transposed HOST-side into lhsT chunk layout [88, nchunks*128] bf16, so
  the gate matmuls read DMA-delivered tiles directly: the PE transpose + DVE
  PSUM-evacuation pipeline for x (~21us DVE + ~14us PE per core) is gone.
- all-tanh gates: sigmoid(z) = (1+tanh(z/2))/2 with the /2 folded into the
  host-packed weights, so each layer's gates need ONE ACT function.  The
  (1+t) corrections ride 4x-rate DVE tensor_scalar adds; the *0.5 rides the
  ACT scale of tanh(c) and host-side W1/W_lin scaling.
- supertile = 64 blocks (PSUM f32 gates), but ACT/DVE instructions are
  merged across supertiles: gate-tanh L0 spans a 2-supertile persistent
  PSUM pair tile [128,2048]; tc1/tc2/y and all elementwise DVE ops span a
  4-supertile quad (256 blocks) in persistent SBUF tiles.  ACT: 35us/core
  transcendental floor + ~35 instrs * 185ns overhead ~= 42us busy.
- 3-stage software pipeline over quads: alpha(r)=L0, beta(r-1)=L1,
  gamma(r-2)=head, persistent tiles ping-pong by r%2.
- per-block row mapping: local row = p*1024 + j (partition-major), so x
  loads and y stores are contiguous >=512B runs per partition.

- nonzero h0/c0 (never produced by the spec) falls back to exact numpy.
"""

import os
import numpy as np
import ml_dtypes

B = 1 << 20
IN_DIM, HID, OUT_DIM = 21, 5, 1
NCORES = 8
BC = B // NCORES          # 131072 rows per core
NBLK = BC // 128          # 1024 blocks per core
NB = 64                   # blocks per supertile
NSUP = NBLK // NB         # 16 supertiles
NQ = NSUP // 4            # 4 quads (256 blocks each)
L0C = 4                   # L0 blocks per chunk: K=88, N=60
L1C = 16                  # L1 blocks per chunk: K=96, N=240
NCH0 = NBLK // L0C        # 256 L0 chunks per core
SCH0 = NB // L0C          # 16 L0 chunks per supertile
SCH1 = NB // L1C          # 4 L1 chunks per supertile
QW = 256 * HID            # 1280: quad width in h-elements
CW = 128 + 60 + 240 + QW  # cpack cols: ident | w0blk | w1blk | wrep

_CACHE = {}
LAST_RESULTS = None


def _build_program(reps=1):
    import contextlib
    import concourse.bacc as bacc
    import concourse.tile as tile
    import concourse.mybir as mybir

    AF = mybir.ActivationFunctionType
    ALU = mybir.AluOpType
    BF16 = mybir.dt.bfloat16
    F32 = mybir.dt.float32
    nc = bacc.Bacc("TRN2", target_bir_lowering=False, debug=False, num_devices=NCORES)

    xt_d = nc.declare_dram_parameter("xt", [88, NCH0 * 128], BF16, isOutput=False)
    cp_d = nc.declare_dram_parameter("cpack", [128, CW], BF16, isOutput=False)
    bl_d = nc.declare_dram_parameter("blin", [128, 1], F32, isOutput=False)
    y_d = nc.declare_dram_parameter("y", [BC, 1], F32, isOutput=True)

    env = lambda k, d: int(os.environ.get(k, d))
    with tile.TileContext(nc) as tc:
        with (
            tc.tile_pool(name="const", bufs=1) as constp,
            tc.tile_pool(name="xin", bufs=env("XIN_BUFS", 4)) as xinp,
            tc.tile_pool(name="g0_ps", bufs=env("G0_BUFS", 1), space="PSUM") as g0psp,
            tc.tile_pool(name="h1t_ps", bufs=env("H1T_BUFS", 2), space="PSUM") as h1tpsp,
            tc.tile_pool(name="g1_ps", bufs=env("G1_BUFS", 1), space="PSUM") as g1psp,
            tc.tile_pool(name="acts", bufs=env("ACTS_BUFS", 2)) as actsp,
        ):
            cp_sb = constp.tile([128, CW], BF16)
            nc.sync.dma_start(cp_sb[:], cp_d[:])
            id_sb = cp_sb[:, 0:128]
            w0_sb = cp_sb[0:88, 128:188]
            w1_sb = cp_sb[0:96, 188:428]
            wr_sb = cp_sb[:, 428 : 428 + QW]
            bl_sb = constp.tile([128, 1], F32)
            nc.gpsimd.dma_start(bl_sb[:], bl_d[:])
            # pre-trigger the tanh ACT table load so its ~2.7us overlaps the
            # first x-load instead of stalling the first gate activation
            warm = constp.tile([128, 1], BF16, tag="actwarm")
            nc.scalar.activation(warm[:, 0:1], id_sb[:, 0:1], AF.Tanh)

            # persistent quad tiles, ping-pong x2 by round parity
            mk2 = lambda nm, w, dt=BF16: [
                constp.tile([128, w], dt, tag=f"{nm}{i}", name=nm) for i in range(2)
            ]
            tg0q = mk2("tg0q", 256 * 15)      # 3840: quad gate-tanh L0
            uv0q = mk2("uv0q", 256 * 10)      # 2560: (1+t) of i|o, L0
            c1q = mk2("c1q", QW)
            tc1q = mk2("tc1q", QW)
            h1q = mk2("h1q", 256 * 6)         # 1536: [h1'(5) | 1] per block
            tg1q = mk2("tg1q", 256 * 15)
            uv1q = mk2("uv1q", 256 * 10)
            c2q = mk2("c2q", QW)
            tc2q = mk2("tc2q", QW)
            for ht in h1q:
                nc.vector.memset(
                    ht[:].rearrange("p (j f) -> p j f", f=6)[:, :, 5:6], 1.0
                )

            if reps > 1:
                rep_ctx = tc.For_i(0, reps, 1, hint_engines=tuple(nc.engines))
            else:
                rep_ctx = contextlib.nullcontext()

            def alpha(r):
                """quad r: x-loads, L0 matmuls into pair PSUM, pair gate-tanh,
                quad uv0/c1/tc1."""
                pp = r % 2
                g0pair = None
                for j in range(4):
                    s = 4 * r + j
                    xt_t = xinp.tile([88, 2048], BF16, tag="xin", name="xt_t")
                    nc.sync.dma_start(
                        out=xt_t[:], in_=xt_d[:, s * 2048 : (s + 1) * 2048]
                    )
                    half = j % 2
                    if half == 0:
                        g0pair = g0psp.tile([128, 2048], F32, tag="g0", name="g0pair")
                    for c in range(SCH0):
                        off = half * 1024 + (c // 8) * 512 + (c % 8) * 60
                        nc.tensor.matmul(
                            g0pair[:, off : off + 60],
                            xt_t[:, c * 128 : (c + 1) * 128],
                            w0_sb[:],
                            start=True,
                            stop=True,
                        )
                    if half == 1:
                        g0v = (
                            g0pair[:]
                            .rearrange("p (b x) -> p b x", x=512)[:, :, 0:480]
                            .rearrange("p b (c n) -> p b c n", n=60)
                        )
                        nc.scalar.activation(
                            tg0q[pp][:, (j // 2) * 1920 : (j // 2 + 1) * 1920]
                            .rearrange("p (b c n) -> p b c n", b=4, c=8, n=60),
                            g0v,
                            AF.Tanh,
                        )
                tg0v = tg0q[pp][:].rearrange("p (C n) -> p C n", n=60)  # C=64 chunks
                nc.vector.tensor_scalar_add(
                    uv0q[pp][:].rearrange("p (C n) -> p C n", n=40),
                    tg0v[:, :, 0:40],
                    1.0,
                )
                uv0v = uv0q[pp][:].rearrange("p (C n) -> p C n", n=40)
                nc.vector.tensor_mul(
                    c1q[pp][:].rearrange("p (C n) -> p C n", n=20),
                    uv0v[:, :, 0:20],
                    tg0v[:, :, 40:60],
                )
                nc.scalar.activation(tc1q[pp][:], c1q[pp][:], AF.Tanh, scale=0.5)

            def beta(r):
                """quad r: h1 assemble, transposes, L1 matmuls, gate-tanh,
                quad uv1/c2/tc2."""
                pp = r % 2
                nc.vector.tensor_mul(
                    h1q[pp][:].rearrange("p (C d f) -> p C d f", C=64, d=4, f=6)[
                        :, :, :, 0:5
                    ],
                    uv0q[pp][:].rearrange(
                        "p (C g d f) -> p C g d f", C=64, g=2, d=4, f=5
                    )[:, :, 1],
                    tc1q[pp][:].rearrange("p (C d f) -> p C d f", C=64, d=4, f=5),
                )
                for j in range(4):
                    h1t = h1tpsp.tile([96, 512], BF16, tag="h1t", name="h1t")
                    for cl in range(SCH1):
                        nc.tensor.transpose(
                            h1t[:, cl * 128 : (cl + 1) * 128],
                            h1q[pp][:, j * 384 + cl * 96 : j * 384 + (cl + 1) * 96],
                            id_sb[:],
                        )
                    h1tsb = actsp.tile([96, 512], BF16, tag="h1tsb", name="h1tsb")
                    nc.vector.tensor_copy(h1tsb[:], h1t[:])
                    g1 = g1psp.tile([128, 1024], F32, tag="g1", name="g1")
                    for cl in range(SCH1):
                        off = (cl // 2) * 512 + (cl % 2) * 240
                        nc.tensor.matmul(
                            g1[:, off : off + 240],
                            h1tsb[:, cl * 128 : (cl + 1) * 128],
                            w1_sb[:],
                            start=True,
                            stop=True,
                        )
                    g1v = (
                        g1[:]
                        .rearrange("p (b x) -> p b x", x=512)[:, :, 0:480]
                        .rearrange("p b (c n) -> p b c n", n=240)
                    )
                    nc.scalar.activation(
                        tg1q[pp][:, j * 960 : (j + 1) * 960]
                        .rearrange("p (b c n) -> p b c n", b=2, c=2, n=240),
                        g1v,
                        AF.Tanh,
                    )
                tg1v = tg1q[pp][:].rearrange("p (C n) -> p C n", n=240)  # C=16
                nc.vector.tensor_scalar_add(
                    uv1q[pp][:].rearrange("p (C n) -> p C n", n=160),
                    tg1v[:, :, 0:160],
                    1.0,
                )
                uv1v = uv1q[pp][:].rearrange("p (C n) -> p C n", n=160)
                nc.vector.tensor_mul(
                    c2q[pp][:].rearrange("p (C n) -> p C n", n=80),
                    uv1v[:, :, 0:80],
                    tg1v[:, :, 160:240],
                )
                nc.scalar.activation(tc2q[pp][:], c2q[pp][:], AF.Tanh, scale=0.5)

            def gamma(r):
                """quad r: vp, t, reduce, y, store."""
                pp = r % 2
                vp = actsp.tile([128, QW], BF16, tag="vp", name="vp")
                nc.vector.tensor_mul(
                    vp[:].rearrange("p (C d f) -> p C d f", C=16, d=16, f=5),
                    uv1q[pp][:].rearrange(
                        "p (C g d f) -> p C g d f", C=16, g=2, d=16, f=5
                    )[:, :, 1],
                    tc2q[pp][:].rearrange("p (C d f) -> p C d f", C=16, d=16, f=5),
                )
                t = actsp.tile([128, QW], BF16, tag="t", name="t")
                nc.vector.tensor_mul(t[:], vp[:], wr_sb[:])
                ypre = actsp.tile([128, 256], F32, tag="ypre", name="ypre")
                nc.vector.tensor_reduce(
                    ypre[:].rearrange("p (j o) -> p j o", o=1),
                    t[:].rearrange("p (j f) -> p j f", f=HID),
                    mybir.AxisListType.X,
                    ALU.add,
                )
                y_tile = actsp.tile([128, 256], F32, tag="y", name="y_tile")
                nc.scalar.activation(y_tile[:], ypre[:], AF.Tanh, bias=bl_sb[:, 0:1])
                nc.gpsimd.dma_start(
                    out=y_d[:].rearrange("(p j) o -> p (j o)", p=128)[
                        :, r * 256 : (r + 1) * 256
                    ],
                    in_=y_tile[:],
                )

            with rep_ctx:
                for r in range(NQ + 2):
                    if r < NQ:
                        alpha(r)
                    if 1 <= r <= NQ:
                        beta(r - 1)
                    if r >= 2:
                        gamma(r - 2)

    nc.compile()
    return nc


def _build_inputs(x, W_ih0, W_hh0, b_ih0, b_hh0, W_ih1, W_hh1, b_ih1, b_hh1, W_lin, b_lin):
    bf16 = ml_dtypes.bfloat16
    b0 = (np.asarray(b_ih0) + np.asarray(b_hh0)).astype(np.float32)
    b1 = (np.asarray(b_ih1) + np.asarray(b_hh1)).astype(np.float32)
    W0 = np.asarray(W_ih0, np.float32)
    W1 = np.asarray(W_ih1, np.float32)
    sel = {"i": range(0, 5), "g": range(10, 15), "o": range(15, 20)}
    # all-tanh gates: sigmoid(z) = (1+tanh(z/2))/2 -> halve i/o gate args
    cs = {"i": 0.5, "o": 0.5, "g": 1.0}

    def blockdiag(W, b, chunk, slot, wx):
        # rows: d*slot + k (k < kin: weights*cs*wx, k == kin: bias*cs)
        kin = W.shape[1]
        out = np.zeros((chunk * slot, chunk * 15), np.float32)
        for d in range(chunk):
            for grp, key in enumerate(("i", "o", "g")):
                for kk, gr in enumerate(sel[key]):
                    col = grp * (chunk * 5) + d * 5 + kk
                    r0 = d * slot
                    out[r0 : r0 + kin, col] = W[gr, :] * cs[key] * wx
                    out[r0 + kin, col] = b[gr] * cs[key]
        return out.astype(bf16)

    w0blk = blockdiag(W0, b0, L0C, 22, 1.0)
    w1blk = blockdiag(W1, b1, L1C, 6, 0.5)  # h1' = 2*h1
    wrep = (
        np.tile(np.asarray(W_lin, np.float32)[0] * 0.5, 256 * 128)  # vp' = 2*h2
        .reshape(128, QW)
        .astype(bf16)
    )
    blin = np.full((128, 1), float(np.asarray(b_lin)[0]), np.float32)
    ident = np.eye(128, dtype=bf16)
    cpack = np.zeros((128, CW), bf16)
    cpack[:, 0:128] = ident
    cpack[0:88, 128:188] = w0blk
    cpack[0:96, 188:428] = w1blk
    cpack[:, 428:] = wrep

    xb = np.empty((B, 22), bf16)
    xb[:, :21] = np.asarray(x, np.float32).astype(bf16)
    xb[:, 21] = bf16(1.0)

    in_maps = []
    for c in range(NCORES):
        xc = xb[c * BC : (c + 1) * BC].reshape(128, NCH0, L0C, 22)
        xt = np.ascontiguousarray(xc.transpose(2, 3, 1, 0)).reshape(88, NCH0 * 128)
        in_maps.append({"xt": xt, "cpack": cpack, "blin": blin})
    return in_maps


def _reference_numpy(x, h0, c0, W_ih0, W_hh0, b_ih0, b_hh0, W_ih1, W_hh1, b_ih1, b_hh1, W_lin, b_lin):
    # general fallback (never taken for the spec'd zero-state inputs)
    def cell(x_, h, c, Wi, Wh, bi, bh):
        g = x_ @ Wi.T + h @ Wh.T + (bi + bh)
        i, f, gg, o = np.split(g, 4, axis=-1)
        sig = lambda z: 1.0 / (1.0 + np.exp(-z))
        cn = sig(f) * c + sig(i) * np.tanh(gg)
        return sig(o) * np.tanh(cn), cn

    h1, _ = cell(x, h0[0], c0[0], W_ih0, W_hh0, b_ih0, b_hh0)
    h2, _ = cell(h1, h0[1], c0[1], W_ih1, W_hh1, b_ih1, b_hh1)
    return np.tanh(h2 @ W_lin.T + b_lin).astype(np.float32)


def kernel(x, h0, c0, W_ih0, W_hh0, b_ih0, b_hh0, W_ih1, W_hh1, b_ih1, b_hh1, W_lin, b_lin):
    global LAST_RESULTS
    args = dict(
        x=np.asarray(x), h0=np.asarray(h0), c0=np.asarray(c0),
        W_ih0=np.asarray(W_ih0), W_hh0=np.asarray(W_hh0),
        b_ih0=np.asarray(b_ih0), b_hh0=np.asarray(b_hh0),
        W_ih1=np.asarray(W_ih1), W_hh1=np.asarray(W_hh1),
        b_ih1=np.asarray(b_ih1), b_hh1=np.asarray(b_hh1),
        W_lin=np.asarray(W_lin), b_lin=np.asarray(b_lin),
    )
    if np.any(args["h0"]) or np.any(args["c0"]):
        return _reference_numpy(**args)

    from concourse.bass_utils import run_bass_kernel_spmd

    if "nc" not in _CACHE:
        _CACHE["nc"] = _build_program()
    nc = _CACHE["nc"]

    in_maps = _build_inputs(
        args["x"], args["W_ih0"], args["W_hh0"], args["b_ih0"], args["b_hh0"],
        args["W_ih1"], args["W_hh1"], args["b_ih1"], args["b_hh1"],
        args["W_lin"], args["b_lin"],
    )
    trace = bool(int(os.environ.get("TRN_TRACE", "0")))
    res = run_bass_kernel_spmd(nc, in_maps, list(range(NCORES)), trace=trace)
    LAST_RESULTS = res
    return np.concatenate([res.results[i]["y"] for i in range(NCORES)], axis=0)
